# revision 9
# baseline (speedup 1.0000x reference)
"""LoRA Linear kernel for Trainium2, 8 NeuronCores, data-parallel over tokens.

out = x @ W^T + bias + 2.0 * (x @ A^T) @ B^T
  x: [4, 2048, 4096] f32, W: [4096, 4096], bias: [4096], A: [16, 4096], B: [4096, 16]

Strategy:
  - Host folds the rank-16 LoRA update into the weight: W' = W + 2*B@A, so the
    device does a single dense GEMM + bias. Bias is fused into the PSUM->SBUF
    drain on the vector engine (tensor_scalar_add with a per-partition scalar).
  - Flatten tokens (8192) and shard 1024 tokens per core (pure data parallel,
    no collectives; gather on host).
  - Mixed precision along the contraction dim: KB of 32 k-tiles run bf16
    (1 col/cycle), the last 2*KQ k-tiles run fp8e4 in DoubleRow pairs (2
    MACs/cell/cycle, K=256 per pass). Both accumulate into the same fp32
    PSUM group (operands quantized UNSCALED so no scale mismatch). Error
    scales as sqrt(2*KQ/32)*3.4e-2 (numpy); HW measures ~0.8x of that.
  - Host pre-arranges operands so every DMA is contiguous per partition:
      xt  [128, KB, 1024]   bf16: xt[p, k, m]     = x_shard^T[k*128+p, m]
      xt8 [128, KQ, 2, 1024] fp8: xt8[p, q, j, m] = x_shard^T[(KB+2q+j)*128+p, m]
      wt  [32, 128, KB, 128]   bf16: wt[oi, p, k, o]     = W'^T[k*128+p, oi*128+o]
      wt8 [32, 128, KQ, 2, 128] fp8: wt8[oi, p, q, j, o] = W'^T[(KB+2q+j)*128+p, oi*128+o]
  - Each core computes out^T [4096, 1024] in [o=128, m=1024] PSUM pair-tiles
    (2 banks). Per m-chunk of 512: 26 bf16 K=128 matmuls + 3 DoubleRow fp8
    K=256 matmuls. One DVE drain + one store per o-tile.
  - Ramp: the first NRAMP o-tiles run unit-outer with staggered starts, so
    the PE consumes x chunks at the DMA frontier instead of waiting for the
    full x^T load. x streams on the scalar HWDGE ring in k-order and W on
    the sync ring (the only two hardware DGE rings; SWDGE via gpsimd steals
    SDMA bandwidth for late-needed bytes and starves the ramp).
  - Single SBUF pool + single PSUM pool: each pool release costs a 5-engine
    barrier round in the epilogue (~0.5 us each).
"""

import sys
from contextlib import ExitStack

import numpy as np

sys.path.insert(0, "/opt/trn_rl_repo")

import concourse.bacc as bacc  # noqa: E402
import concourse.bass as bass  # noqa: E402
import concourse.mybir as mybir  # noqa: E402
import concourse.tile as tile  # noqa: E402
from concourse.bass import ts  # noqa: E402
from concourse.bass_utils import run_bass_kernel_spmd  # noqa: E402

from ml_dtypes import bfloat16, float8_e4m3  # noqa: E402

P = 128
B_DIM, S_DIM = 4, 2048
D = 4096          # in_features (contraction)
O = 4096          # out_features
R = 16            # lora rank
SCALING = 2.0     # alpha / rank = 32/16
NCORES = 8
M = (B_DIM * S_DIM) // NCORES   # tokens per core = 1024
KD = D // P       # 32 contraction tiles
KB = 20           # bf16 k-tiles (logical k-tiles 2*KQ..31)
KQ = 6            # fp8 DoubleRow pairs (logical k-tiles 0..2*KQ-1)
NU = KB + KQ      # 29 accumulation units per (o, m) group
MC = 512          # moving free dim per matmul (one PSUM bank of fp32)
NMC = M // MC     # 2 m-chunks
NO = O // P       # 32 output-feature tiles
NRAMP = 4         # o-tiles unit-outer-interleaved during the x-load ramp
STRIDE = 2        # unit-slot stagger between consecutive ramp tiles
NWARM = 75        # tiny const matmuls to warm the HAM clock gate

BF = mybir.dt.bfloat16
F8 = mybir.dt.float8e4
F32 = mybir.dt.float32
DR = mybir.MatmulPerfMode.DoubleRow


def build_program() -> bass.Bass:
    # Bacc (not plain Bass): its compile() pipeline moves extra matmul waits
    # onto LDWEIGHTS and splits any remainder via event semaphores.
    nc = bacc.Bacc()
    xt = nc.dram_tensor("xt", [P, KB, M], BF, kind="ExternalInput")
    xt8 = nc.dram_tensor("xt8", [P, KQ, 2, M], F8, kind="ExternalInput")
    wt = nc.dram_tensor("wt", [NO, P, KB, P], BF, kind="ExternalInput")
    wt8 = nc.dram_tensor("wt8", [NO, P, KQ, 2, P], F8, kind="ExternalInput")
    bs = nc.dram_tensor("bs", [P, NO], F32, kind="ExternalInput")
    outT = nc.dram_tensor("outT", [O, M], BF, kind="ExternalOutput")

    with ExitStack() as ctx:
        tc = ctx.enter_context(tile.TileContext(nc))
        pool = ctx.enter_context(tc.tile_pool(name="sb", bufs=1))
        ps_pool = ctx.enter_context(tc.tile_pool(name="psp", bufs=4, space="PSUM"))

        xt_sb = pool.tile([P, KB, M], BF)
        xt8_sb = pool.tile([P, KQ, 2, M], F8)
        bias_sb = pool.tile([P, NO], F32)

        def w_load(oi, split=1):
            wt_sb = pool.tile([P, KB, P], BF, name="wt_sb", bufs=4)
            wt8_sb = pool.tile([P, KQ, 2, P], F8, name="wt8_sb", bufs=4)
            nc.sync.dma_start(wt8_sb[:], wt8[oi])
            for h in range(split):
                k0, k1 = (h * KB) // split, ((h + 1) * KB) // split
                nc.sync.dma_start(wt_sb[:, k0:k1, :], wt[oi, :, k0:k1, :])
            return wt_sb, wt8_sb

        # Ramp loads, emitted in PE need-order (units run fp8 DR pairs
        # FIRST: the fp8 x is half the bytes of bf16, so the early HBM burst
        # is smaller and the bf16 x frontier gets ~6 extra slots of slack).
        # W on the sync HWDGE ring, x on the scalar ring; the SDMA engines
        # round-robin packets between the two rings (fair bandwidth share).
        ramp_wt = [
            (
                pool.tile([P, KB, P], BF, name="wt_sb", bufs=4),
                pool.tile([P, KQ, 2, P], F8, name="wt8_sb", bufs=4),
            )
            for _ in range(NRAMP)
        ]
        for t in range(NRAMP):
            nc.sync.dma_start(ramp_wt[t][1][:], wt8[t])
        for h in range(2):
            k0, k1 = (h * KB) // 2, ((h + 1) * KB) // 2
            for t in range(NRAMP):
                nc.sync.dma_start(
                    ramp_wt[t][0][:, k0:k1, :], wt[t, :, k0:k1, :]
                )
        # x on the scalar ring: fp8 pairs first (one dispatch per DR unit so
        # the first real matmul only waits on a 256KB transfer), then the
        # bf16 chunks in k-order, then bias (needed only at the first drain).
        for q in range(KQ):
            nc.scalar.dma_start(xt8_sb[:, q], xt8[:, q])
        k0 = 0
        for kc in (1, 1) + (2,) * ((KB - 2) // 2):
            nc.scalar.dma_start(xt_sb[:, k0 : k0 + kc, :], xt[:, k0 : k0 + kc, :])
            k0 += kc
        nc.scalar.dma_start(bias_sb[:], bs[:])

        def drain(oi, ps):
            ot = pool.tile([P, M], BF, name="ot", bufs=4)
            nc.vector.tensor_scalar_add(ot[:], ps[:], bias_sb[:, oi : oi + 1])
            nc.scalar.dma_start(outT[ts(oi, P), :], ot[:])

        def mm_unit(ps, w_tiles, u, mi, start, stop, skip=False, pmi=None):
            """One accumulation unit: fp8 DR pair (u<KQ) or bf16 k-tile.

            `mi` selects the x m-chunk; `pmi` (default mi) selects which
            PSUM column slot of `ps` accumulates it."""
            wt_sb, wt8_sb = w_tiles
            pmi = mi if pmi is None else pmi
            if u < KQ:
                nc.tensor.matmul(
                    ps[:, ts(pmi, MC)],
                    lhsT=wt8_sb[:, u],
                    rhs=xt8_sb[:, u, :, ts(mi, MC)],
                    start=start,
                    stop=stop,
                    perf_mode=DR,
                    skip_group_check=skip,
                )
            else:
                k = u - KQ
                nc.tensor.matmul(
                    ps[:, ts(pmi, MC)],
                    lhsT=wt_sb[:, k, :],
                    rhs=xt_sb[:, k, ts(mi, MC)],
                    start=start,
                    stop=stop,
                    skip_group_check=skip,
                )

        def mm_pair(ps, w_tiles, u, start, stop):
            for mi in range(NMC):
                mm_unit(ps, w_tiles, u, mi, start, stop, skip=True)

        # Ramp: NRAMP o-tiles advance together along the x-chunk frontier;
        # tile t joins at unit STRIDE*t and wraps to finish its first unit
        # last. Same "ps" ring tag as the steady loop: 4 pair-tiles = all 8
        # banks; steady allocations wrap the ring and wait on ramp drains.
        ramp_ps = [
            ps_pool.tile([P, M], F32, name="ps") for _ in range(NRAMP)
        ]

        # Warm-up: tiny N=1 matmuls on the framework's const tile (memset in
        # the preamble block, so no DMA/DVE dependency at all) keep the PE
        # busy from ~6.4 us so the HAM clock-gate is at 8/8 (2.4 GHz) when
        # the first real matmul's data lands. ~50ns each (NX dispatch floor);
        # the first real start=True clears the scratch bank.
        cb = nc.const_aps.aps[(mybir.dt.bfloat16, 1.0)]
        for i in range(NWARM):
            nc.tensor.matmul(
                ramp_ps[0][0:1, 0:1],
                lhsT=cb,
                rhs=cb,
                start=(i == 0),
                stop=False,
                skip_group_check=True,
            )
        for s in range(NU + STRIDE * (NRAMP - 1)):
            for t in range(NRAMP):
                if s < STRIDE * t or s >= STRIDE * t + NU:
                    continue
                u = s if s < NU else s - NU
                mm_pair(
                    ramp_ps[t],
                    ramp_wt[t],
                    u,
                    start=(s == STRIDE * t),
                    stop=(s == STRIDE * t + NU - 1),
                )
            for t in range(NRAMP):
                if s == STRIDE * t + NU - 1:
                    drain(t, ramp_ps[t])

        # Steady state: one o-tile at a time, W blocks prefetched 3 deep.
        # The last tile drains per m-chunk (mi=0 drain overlaps mi=1's
        # matmuls; final store is half-size) to shorten the tail.
        for oi in range(NRAMP, NO):
            w_tiles = w_load(oi)
            last = oi == NO - 1
            if not last:
                ps = ps_pool.tile([P, M], F32, name="ps")
                for mi in range(NMC):
                    for u in range(NU):
                        mm_unit(
                            ps, w_tiles, u, mi, start=(u == 0), stop=(u == NU - 1)
                        )
                drain(oi, ps)
            else:
                # Last tile: each m-chunk accumulates in its OWN PSUM ring
                # slot (deps are tile-granular, so with a shared tile the
                # mi=0 drain and the mi=1 matmuls serialize ~1.5us onto the
                # kernel tail). The mi=0 drain overlaps the mi=1 matmuls;
                # the final store is split across both rings so the HBM
                # write receipts (~2 us each) overlap instead of stacking.
                for mi in range(NMC):
                    ps = ps_pool.tile([P, M], F32, name="ps")
                    for u in range(NU):
                        mm_unit(
                            ps, w_tiles, u, mi,
                            start=(u == 0), stop=(u == NU - 1), pmi=0,
                        )
                    ot = pool.tile([P, MC], BF, name=f"lot{mi}", bufs=1)
                    nc.vector.tensor_scalar_add(
                        ot[:], ps[:, 0:MC], bias_sb[:, oi : oi + 1]
                    )
                    if mi == 0:
                        nc.scalar.dma_start(outT[ts(oi, P), ts(mi, MC)], ot[:])
                    else:
                        h = MC // 2
                        nc.sync.dma_start(
                            outT[ts(oi, P), mi * MC : mi * MC + h], ot[:, 0:h]
                        )
                        nc.scalar.dma_start(
                            outT[ts(oi, P), mi * MC + h : (mi + 1) * MC],
                            ot[:, h:MC],
                        )
    nc.compile()
    return nc


def prepare_in_maps(inputs, weight, bias, lora_a, lora_b):
    w_eff = np.asarray(weight, dtype=np.float32) + SCALING * (
        np.asarray(lora_b, dtype=np.float32) @ np.asarray(lora_a, dtype=np.float32)
    )
    # wT_r[k, p, oi, o] = W'^T[k*128+p, oi*128+o]
    wT_r = w_eff.T.reshape(KD, P, NO, P)
    # wt[oi, p, k, o]: contiguous per-partition blocks, bf16 k-tiles 0..25
    wt = np.ascontiguousarray(wT_r[:KB].transpose(2, 1, 0, 3)).astype(bfloat16)
    # wt8[oi, p, q, j, o]: fp8 k-tiles 26..31 as DoubleRow pairs
    wt8 = np.ascontiguousarray(
        wT_r[KB:].reshape(KQ, 2, P, NO, P).transpose(3, 2, 0, 1, 4)
    ).astype(float8_e4m3)
    bs = np.ascontiguousarray(np.asarray(bias, dtype=np.float32).reshape(NO, P).T)
    x = np.asarray(inputs, dtype=np.float32).reshape(B_DIM * S_DIM, D)
    in_maps = []
    for c in range(NCORES):
        xT_r = x[c * M : (c + 1) * M].T.reshape(KD, P, M)
        # xt[p, k, m] = x_shard^T[k*128+p, m]
        xt_c = np.ascontiguousarray(xT_r[:KB].transpose(1, 0, 2)).astype(bfloat16)
        xt8_c = np.ascontiguousarray(
            xT_r[KB:].reshape(KQ, 2, P, M).transpose(2, 0, 1, 3)
        ).astype(float8_e4m3)
        in_maps.append({"xt": xt_c, "xt8": xt8_c, "wt": wt, "wt8": wt8, "bs": bs})
    return in_maps


def run(inputs, weight, bias, lora_a, lora_b, trace=False):
    nc = build_program()
    in_maps = prepare_in_maps(inputs, weight, bias, lora_a, lora_b)
    res = run_bass_kernel_spmd(nc, in_maps, list(range(NCORES)), trace=trace)
    shards = [
        np.asarray(res.results[c]["outT"]).astype(np.float32).T
        for c in range(NCORES)
    ]
    out = np.concatenate(shards, axis=0).reshape(B_DIM, S_DIM, O)
    return np.ascontiguousarray(out, dtype=np.float32), res


def kernel(inputs, weight, bias, lora_a, lora_b):
    out, _ = run(inputs, weight, bias, lora_a, lora_b, trace=False)
    return out


# revision 10
# speedup vs baseline: 1.0191x; 1.0191x over previous
"""LoRA Linear kernel for Trainium2, 8 NeuronCores, data-parallel over tokens.

out = x @ W^T + bias + 2.0 * (x @ A^T) @ B^T
  x: [4, 2048, 4096] f32, W: [4096, 4096], bias: [4096], A: [16, 4096], B: [4096, 16]

Strategy:
  - Host folds the rank-16 LoRA update into the weight: W' = W + 2*B@A, so the
    device does a single dense GEMM + bias. Bias is fused into the PSUM->SBUF
    drain on the vector engine (tensor_scalar_add with a per-partition scalar).
  - Flatten tokens (8192) and shard 1024 tokens per core (pure data parallel,
    no collectives; gather on host).
  - Mixed precision along the contraction dim: KB of 32 k-tiles run bf16
    (1 col/cycle), the last 2*KQ k-tiles run fp8e4 in DoubleRow pairs (2
    MACs/cell/cycle, K=256 per pass). Both accumulate into the same fp32
    PSUM group (operands quantized UNSCALED so no scale mismatch). Error
    scales as sqrt(2*KQ/32)*3.4e-2 (numpy); HW measures ~0.8x of that.
  - Host pre-arranges operands so every DMA is contiguous per partition:
      xt  [128, KB, 1024]   bf16: xt[p, k, m]     = x_shard^T[k*128+p, m]
      xt8 [128, KQ, 2, 1024] fp8: xt8[p, q, j, m] = x_shard^T[(KB+2q+j)*128+p, m]
      wt  [32, 128, KB, 128]   bf16: wt[oi, p, k, o]     = W'^T[k*128+p, oi*128+o]
      wt8 [32, 128, KQ, 2, 128] fp8: wt8[oi, p, q, j, o] = W'^T[(KB+2q+j)*128+p, oi*128+o]
  - Each core computes out^T [4096, 1024] in [o=128, m=1024] PSUM pair-tiles
    (2 banks). Per m-chunk of 512: 26 bf16 K=128 matmuls + 3 DoubleRow fp8
    K=256 matmuls. One DVE drain + one store per o-tile.
  - Ramp: the first NRAMP o-tiles run unit-outer with staggered starts, so
    the PE consumes x chunks at the DMA frontier instead of waiting for the
    full x^T load. x streams on the scalar HWDGE ring in k-order and W on
    the sync ring (the only two hardware DGE rings; SWDGE via gpsimd steals
    SDMA bandwidth for late-needed bytes and starves the ramp).
  - Single SBUF pool + single PSUM pool: each pool release costs a 5-engine
    barrier round in the epilogue (~0.5 us each).
"""

import sys
from contextlib import ExitStack

import numpy as np

sys.path.insert(0, "/opt/trn_rl_repo")

import concourse.bacc as bacc  # noqa: E402
import concourse.bass as bass  # noqa: E402
import concourse.mybir as mybir  # noqa: E402
import concourse.tile as tile  # noqa: E402
from concourse.bass import ts  # noqa: E402
from concourse.bass_utils import run_bass_kernel_spmd  # noqa: E402

from ml_dtypes import bfloat16, float8_e4m3  # noqa: E402

P = 128
B_DIM, S_DIM = 4, 2048
D = 4096          # in_features (contraction)
O = 4096          # out_features
R = 16            # lora rank
SCALING = 2.0     # alpha / rank = 32/16
NCORES = 8
M = (B_DIM * S_DIM) // NCORES   # tokens per core = 1024
KD = D // P       # 32 contraction tiles
KB = 20           # bf16 k-tiles (logical k-tiles 2*KQ..31)
KQ = 6            # fp8 DoubleRow pairs (logical k-tiles 0..2*KQ-1)
NU = KB + KQ      # 29 accumulation units per (o, m) group
MC = 512          # moving free dim per matmul (one PSUM bank of fp32)
NMC = M // MC     # 2 m-chunks
NO = O // P       # 32 output-feature tiles
NRAMP = 4         # o-tiles unit-outer-interleaved during the x-load ramp
STRIDE = 0        # no stagger: all ramp tiles advance together (every
                  # slot = NRAMP*NMC matmuls per 256KB x-chunk, so the
                  # ramp's byte demand stays ~220GB/s from slot 0)
NWARM = 75        # tiny const matmuls to warm the HAM clock gate

BF = mybir.dt.bfloat16
F8 = mybir.dt.float8e4
F32 = mybir.dt.float32
DR = mybir.MatmulPerfMode.DoubleRow


def build_program() -> bass.Bass:
    # Bacc (not plain Bass): its compile() pipeline moves extra matmul waits
    # onto LDWEIGHTS and splits any remainder via event semaphores.
    nc = bacc.Bacc()
    xt = nc.dram_tensor("xt", [P, KB, M], BF, kind="ExternalInput")
    xt8 = nc.dram_tensor("xt8", [P, KQ, 2, M], F8, kind="ExternalInput")
    wt = nc.dram_tensor("wt", [NO, P, KB, P], BF, kind="ExternalInput")
    wt8 = nc.dram_tensor("wt8", [NO, P, KQ, 2, P], F8, kind="ExternalInput")
    bs = nc.dram_tensor("bs", [P, NO], F32, kind="ExternalInput")
    outT = nc.dram_tensor("outT", [O, M], BF, kind="ExternalOutput")

    with ExitStack() as ctx:
        tc = ctx.enter_context(tile.TileContext(nc))
        pool = ctx.enter_context(tc.tile_pool(name="sb", bufs=1))
        ps_pool = ctx.enter_context(tc.tile_pool(name="psp", bufs=4, space="PSUM"))

        xt_sb = pool.tile([P, KB, M], BF)
        xt8_sb = pool.tile([P, KQ, 2, M], F8)
        bias_sb = pool.tile([P, NO], F32)

        def w_load(oi, split=1):
            wt_sb = pool.tile([P, KB, P], BF, name="wt_sb", bufs=4)
            wt8_sb = pool.tile([P, KQ, 2, P], F8, name="wt8_sb", bufs=4)
            nc.sync.dma_start(wt8_sb[:], wt8[oi])
            for h in range(split):
                k0, k1 = (h * KB) // split, ((h + 1) * KB) // split
                nc.sync.dma_start(wt_sb[:, k0:k1, :], wt[oi, :, k0:k1, :])
            return wt_sb, wt8_sb

        # Ramp loads, emitted in PE need-order (units run fp8 DR pairs
        # FIRST: the fp8 x is half the bytes of bf16, so the early HBM burst
        # is smaller and the bf16 x frontier gets ~6 extra slots of slack).
        # W on the sync HWDGE ring, x on the scalar ring; the SDMA engines
        # round-robin packets between the two rings (fair bandwidth share).
        ramp_wt = [
            (
                pool.tile([P, KB, P], BF, name="wt_sb", bufs=4),
                pool.tile([P, KQ, 2, P], F8, name="wt8_sb", bufs=4),
            )
            for _ in range(NRAMP)
        ]
        for t in range(NRAMP):
            nc.sync.dma_start(ramp_wt[t][1][:], wt8[t])
        for h in range(2):
            k0, k1 = (h * KB) // 2, ((h + 1) * KB) // 2
            for t in range(NRAMP):
                nc.sync.dma_start(
                    ramp_wt[t][0][:, k0:k1, :], wt[t, :, k0:k1, :]
                )
        # x on the scalar ring: fp8 pairs first (one dispatch per DR unit so
        # the first real matmul only waits on a 256KB transfer), then the
        # bf16 chunks in k-order, then bias (needed only at the first drain).
        for q in range(KQ):
            nc.scalar.dma_start(xt8_sb[:, q], xt8[:, q])
        k0 = 0
        for kc in (1, 1) + (2,) * ((KB - 2) // 2):
            nc.scalar.dma_start(xt_sb[:, k0 : k0 + kc, :], xt[:, k0 : k0 + kc, :])
            k0 += kc
        nc.scalar.dma_start(bias_sb[:], bs[:])

        def drain(oi, ps):
            ot = pool.tile([P, M], BF, name="ot", bufs=4)
            nc.vector.tensor_scalar_add(ot[:], ps[:], bias_sb[:, oi : oi + 1])
            nc.scalar.dma_start(outT[ts(oi, P), :], ot[:])

        def mm_unit(ps, w_tiles, u, mi, start, stop, skip=False, pmi=None):
            """One accumulation unit: fp8 DR pair (u<KQ) or bf16 k-tile.

            `mi` selects the x m-chunk; `pmi` (default mi) selects which
            PSUM column slot of `ps` accumulates it."""
            wt_sb, wt8_sb = w_tiles
            pmi = mi if pmi is None else pmi
            if u < KQ:
                nc.tensor.matmul(
                    ps[:, ts(pmi, MC)],
                    lhsT=wt8_sb[:, u],
                    rhs=xt8_sb[:, u, :, ts(mi, MC)],
                    start=start,
                    stop=stop,
                    perf_mode=DR,
                    skip_group_check=skip,
                )
            else:
                k = u - KQ
                nc.tensor.matmul(
                    ps[:, ts(pmi, MC)],
                    lhsT=wt_sb[:, k, :],
                    rhs=xt_sb[:, k, ts(mi, MC)],
                    start=start,
                    stop=stop,
                    skip_group_check=skip,
                )

        def mm_pair(ps, w_tiles, u, start, stop):
            for mi in range(NMC):
                mm_unit(ps, w_tiles, u, mi, start, stop, skip=True)

        # Ramp: NRAMP o-tiles advance together along the x-chunk frontier;
        # tile t joins at unit STRIDE*t and wraps to finish its first unit
        # last. Same "ps" ring tag as the steady loop: 4 pair-tiles = all 8
        # banks; steady allocations wrap the ring and wait on ramp drains.
        ramp_ps = [
            ps_pool.tile([P, M], F32, name="ps") for _ in range(NRAMP)
        ]

        # Warm-up: tiny N=1 matmuls on the framework's const tile (memset in
        # the preamble block, so no DMA/DVE dependency at all) keep the PE
        # busy from ~6.4 us so the HAM clock-gate is at 8/8 (2.4 GHz) when
        # the first real matmul's data lands. ~50ns each (NX dispatch floor);
        # the first real start=True clears the scratch bank.
        cb = nc.const_aps.aps[(mybir.dt.bfloat16, 1.0)]
        for i in range(NWARM):
            nc.tensor.matmul(
                ramp_ps[0][0:1, 0:1],
                lhsT=cb,
                rhs=cb,
                start=(i == 0),
                stop=False,
                skip_group_check=True,
            )
        for s in range(NU + STRIDE * (NRAMP - 1)):
            for t in range(NRAMP):
                if s < STRIDE * t or s >= STRIDE * t + NU:
                    continue
                u = s if s < NU else s - NU
                mm_pair(
                    ramp_ps[t],
                    ramp_wt[t],
                    u,
                    start=(s == STRIDE * t),
                    stop=(s == STRIDE * t + NU - 1),
                )
            for t in range(NRAMP):
                if s == STRIDE * t + NU - 1:
                    drain(t, ramp_ps[t])

        # Steady state: one o-tile at a time, W blocks prefetched 3 deep.
        # The last tile drains per m-chunk (mi=0 drain overlaps mi=1's
        # matmuls; final store is half-size) to shorten the tail.
        for oi in range(NRAMP, NO):
            w_tiles = w_load(oi)
            last = oi == NO - 1
            if not last:
                ps = ps_pool.tile([P, M], F32, name="ps")
                for mi in range(NMC):
                    for u in range(NU):
                        mm_unit(
                            ps, w_tiles, u, mi, start=(u == 0), stop=(u == NU - 1)
                        )
                drain(oi, ps)
            else:
                # Last tile: each m-chunk accumulates in its OWN PSUM ring
                # slot (deps are tile-granular, so with a shared tile the
                # mi=0 drain and the mi=1 matmuls serialize ~1.5us onto the
                # kernel tail). The mi=0 drain overlaps the mi=1 matmuls;
                # the final store is split across both rings so the HBM
                # write receipts (~2 us each) overlap instead of stacking.
                for mi in range(NMC):
                    ps = ps_pool.tile([P, M], F32, name="ps")
                    for u in range(NU):
                        mm_unit(
                            ps, w_tiles, u, mi,
                            start=(u == 0), stop=(u == NU - 1), pmi=0,
                        )
                    ot = pool.tile([P, MC], BF, name=f"lot{mi}", bufs=1)
                    nc.vector.tensor_scalar_add(
                        ot[:], ps[:, 0:MC], bias_sb[:, oi : oi + 1]
                    )
                    if mi == 0:
                        nc.scalar.dma_start(outT[ts(oi, P), ts(mi, MC)], ot[:])
                    else:
                        h = MC // 2
                        nc.sync.dma_start(
                            outT[ts(oi, P), mi * MC : mi * MC + h], ot[:, 0:h]
                        )
                        nc.scalar.dma_start(
                            outT[ts(oi, P), mi * MC + h : (mi + 1) * MC],
                            ot[:, h:MC],
                        )
    nc.compile()
    return nc


def prepare_in_maps(inputs, weight, bias, lora_a, lora_b):
    w_eff = np.asarray(weight, dtype=np.float32) + SCALING * (
        np.asarray(lora_b, dtype=np.float32) @ np.asarray(lora_a, dtype=np.float32)
    )
    # wT_r[k, p, oi, o] = W'^T[k*128+p, oi*128+o]
    wT_r = w_eff.T.reshape(KD, P, NO, P)
    # wt[oi, p, k, o]: contiguous per-partition blocks, bf16 k-tiles 0..25
    wt = np.ascontiguousarray(wT_r[:KB].transpose(2, 1, 0, 3)).astype(bfloat16)
    # wt8[oi, p, q, j, o]: fp8 k-tiles 26..31 as DoubleRow pairs
    wt8 = np.ascontiguousarray(
        wT_r[KB:].reshape(KQ, 2, P, NO, P).transpose(3, 2, 0, 1, 4)
    ).astype(float8_e4m3)
    bs = np.ascontiguousarray(np.asarray(bias, dtype=np.float32).reshape(NO, P).T)
    x = np.asarray(inputs, dtype=np.float32).reshape(B_DIM * S_DIM, D)
    in_maps = []
    for c in range(NCORES):
        xT_r = x[c * M : (c + 1) * M].T.reshape(KD, P, M)
        # xt[p, k, m] = x_shard^T[k*128+p, m]
        xt_c = np.ascontiguousarray(xT_r[:KB].transpose(1, 0, 2)).astype(bfloat16)
        xt8_c = np.ascontiguousarray(
            xT_r[KB:].reshape(KQ, 2, P, M).transpose(2, 0, 1, 3)
        ).astype(float8_e4m3)
        in_maps.append({"xt": xt_c, "xt8": xt8_c, "wt": wt, "wt8": wt8, "bs": bs})
    return in_maps


def run(inputs, weight, bias, lora_a, lora_b, trace=False):
    nc = build_program()
    in_maps = prepare_in_maps(inputs, weight, bias, lora_a, lora_b)
    res = run_bass_kernel_spmd(nc, in_maps, list(range(NCORES)), trace=trace)
    shards = [
        np.asarray(res.results[c]["outT"]).astype(np.float32).T
        for c in range(NCORES)
    ]
    out = np.concatenate(shards, axis=0).reshape(B_DIM, S_DIM, O)
    return np.ascontiguousarray(out, dtype=np.float32), res


def kernel(inputs, weight, bias, lora_a, lora_b):
    out, _ = run(inputs, weight, bias, lora_a, lora_b, trace=False)
    return out
